# revision 1
# baseline (speedup 1.0000x reference)
"""Trainium2 Bass kernel for nn_ExperimentalMSELoss_17935783428185.

Reference math (pred, target: [64, 1, 512, 512] f32, uniform [0,1)):
    mask = target > 0.1
    i    = clip(target*mask, 1e-8)^0.001
    total_map = (pred*mask*i - target*mask*i)^2 + ((pred-target)*(1-mask))^2
              = (pred-target)^2 * (mask*target^0.002 + (1-mask))
    loss = total_map.sum()
         + 1e-3 * sum_b |max_b pred - max_b target| / numel      (~3e-19 rel)
         + 1e-3 * sum_b |sum_b pred - sum_b target| / numel      (~1e-11 rel)
         + 1e-3 * mean((hist10(pred) - hist10(target))^2)        (~2.5e-16 rel)

The three weighted terms are 8+ orders of magnitude below the f32 ULP of the
map-sum (~9e-8 relative); the reference's own f32 addition rounds the max and
hist terms away entirely. We compute the dominant map-sum exactly and the sum
term (it rides free on a fused accumulate); max/hist are omitted - verified to
change the f32 result by < 1e-11 relative.

Device computation, data-parallel over batch (8 samples per core, 8 cores):
    per sample tile [128, 2048]:
      u = max(target, 0.1)                      (GPSIMD tensor_scalar)
      v = ln(u)                                 (ACT)
      p = (target > 0.1) * v                    (DVE scalar_tensor_tensor)
      e = exp(0.001 * p)                        (ACT)          # e^2 = weight
      d = pred - target    [+ accum sum(d)]     (DVE stt)
      g = d * e                                 (DVE tensor_tensor)
      s2[:, s] = sum_free(g^2)                  (ACT Square + accum_out)
    out[128, 16] = [s2 (8 cols) | sd (8 cols)]
Host: f64 reduction of the per-partition stats, final f32 cast.
"""

import numpy as np

B, H, W = 64, 512, 512
NUMEL = H * W                    # 262144 per sample
P, F = 128, NUMEL // 128         # [128, 2048] per-sample tile
N_CORES = 8
SAMPLES_PER_CORE = B // N_CORES  # 8
THRESH = 0.1
EPS = 1e-8

_CACHE = {}


def build_kernel(repeat: int = 1, samples_per_core: int = SAMPLES_PER_CORE):
    """Build + compile the per-core Bass program. `repeat` re-runs the whole
    compute `repeat` times (for wall-clock slope timing); results identical."""
    import concourse.bacc as bacc
    import concourse.mybir as mybir
    import concourse.tile as tile

    S = samples_per_core
    f32 = mybir.dt.float32
    Alu = mybir.AluOpType
    Act = mybir.ActivationFunctionType

    nc = bacc.Bacc("TRN2", target_bir_lowering=False, debug=False)
    pred = nc.dram_tensor("pred", [S, P, F], f32, kind="ExternalInput").ap()
    target = nc.dram_tensor("target", [S, P, F], f32, kind="ExternalInput").ap()
    out = nc.dram_tensor("out", [P, 2 * S], f32, kind="ExternalOutput").ap()

    with tile.TileContext(nc) as tc:
        with (
            tc.tile_pool(name="work", bufs=2) as pool,
            tc.tile_pool(name="stats", bufs=1) as statpool,
        ):
            s2 = statpool.tile([P, S], f32)
            sd = statpool.tile([P, S], f32)
            for _ in range(repeat):
                for s in range(S):
                    a = pool.tile([P, F], f32, tag="a")
                    b = pool.tile([P, F], f32, tag="b")
                    nc.sync.dma_start(out=a, in_=pred[s])
                    nc.sync.dma_start(out=b, in_=target[s])

                    u = pool.tile([P, F], f32, tag="u")
                    nc.gpsimd.tensor_scalar_max(out=u, in0=b, scalar1=THRESH)
                    v = pool.tile([P, F], f32, tag="v")
                    nc.scalar.activation(out=v, in_=u, func=Act.Ln)
                    p = pool.tile([P, F], f32, tag="p")
                    nc.vector.scalar_tensor_tensor(
                        out=p, in0=b, scalar=THRESH, in1=v,
                        op0=Alu.is_gt, op1=Alu.mult,
                    )
                    e = pool.tile([P, F], f32, tag="e")
                    nc.scalar.activation(out=e, in_=p, func=Act.Exp, scale=0.001)

                    d = pool.tile([P, F], f32, tag="d")
                    nc.vector.scalar_tensor_tensor(
                        out=d, in0=a, scalar=0.0, in1=b,
                        op0=Alu.bypass, op1=Alu.subtract,
                        accum_out=sd[:, s : s + 1],
                    )
                    g = pool.tile([P, F], f32, tag="g")
                    nc.vector.tensor_tensor(out=g, in0=d, in1=e, op=Alu.mult)
                    gs = pool.tile([P, F], f32, tag="gs")
                    nc.scalar.activation(
                        out=gs, in_=g, func=Act.Square,
                        accum_out=s2[:, s : s + 1],
                    )
            nc.sync.dma_start(out=out[:, 0:S], in_=s2)
            nc.sync.dma_start(out=out[:, S : 2 * S], in_=sd)

    nc.compile()
    return nc


def _get_kernel(repeat: int = 1):
    key = repeat
    if key not in _CACHE:
        _CACHE[key] = build_kernel(repeat)
    return _CACHE[key]


def run_device(pred: np.ndarray, target: np.ndarray, repeat: int = 1):
    """Shard, run on 8 cores, return list of per-core out [128, 16] arrays."""
    from concourse.bass_utils import run_bass_kernel_spmd

    nc = _get_kernel(repeat)
    pred_rs = np.ascontiguousarray(
        np.asarray(pred, dtype=np.float32).reshape(B, P, F)
    )
    target_rs = np.ascontiguousarray(
        np.asarray(target, dtype=np.float32).reshape(B, P, F)
    )
    S = SAMPLES_PER_CORE
    in_maps = [
        {"pred": pred_rs[c * S : (c + 1) * S], "target": target_rs[c * S : (c + 1) * S]}
        for c in range(N_CORES)
    ]
    res = run_bass_kernel_spmd(nc, in_maps, core_ids=list(range(N_CORES)))
    return [res.results[c]["out"] for c in range(N_CORES)]


def kernel(pred: np.ndarray, target: np.ndarray) -> np.ndarray:
    outs = run_device(pred, target)
    s2_total = 0.0
    abs_sd_total = 0.0
    S = SAMPLES_PER_CORE
    for o in outs:
        o64 = o.astype(np.float64)
        s2_total += o64[:, :S].sum()
        abs_sd_total += np.abs(o64[:, S:].sum(axis=0)).sum()
    total = s2_total + 1e-3 * abs_sd_total / (NUMEL + EPS)
    return np.asarray(total, dtype=np.float32)
